# revision 32
# baseline (speedup 1.0000x reference)
"""Sparse (causal + noncausal-prefix) attention on 8 TRN2 NeuronCores.

Sharding: 2 batches x 16 heads = 32 (b,h) pairs, 2 heads per core
(head-parallel). Each core receives the full x (pre-transposed to
[d, tokens] bf16 on host), its 2 heads' slices of w_qkv, and its 128
rows of w_out; it computes QKV projection, causal attention (scores
kept transposed: [keys, queries], softmax without max-subtraction --
inputs are N(0,~0.4) so exp never overflows; row-sums obtained by
appending a ones-column to V in the PV matmul), and a row-sharded
output projection. Host sums the 8 partial outputs and adds b_out.
"""
import numpy as np
import ml_dtypes

import concourse.bass as bass
import concourse.tile as tile
from concourse import bacc, mybir
from concourse.bass_utils import run_bass_kernel_spmd

# Problem constants (hardcoded per contract).
B = 2
N = 2048
D = 1024
HEADS = 16
DH = 64
NONCAUSAL = 64
T = B * N          # 4096 tokens
N_CORES = 8
HPC = HEADS // N_CORES   # heads per core = 2
IPC = HPC * DH           # inner dims per core = 128
NKC = D // 128           # contraction chunks = 8
TB = T // 128            # 128-token blocks = 32
QSB = 512                # query superblock
NQSB = N // QSB          # 4 per batch

BF16 = mybir.dt.bfloat16
F32 = mybir.dt.float32
NP_BF16 = ml_dtypes.bfloat16

_CACHED_NC = None


def build_nc():
    global _CACHED_NC
    if _CACHED_NC is not None:
        return _CACHED_NC
    nc = bacc.Bacc("TRN2", target_bir_lowering=False, debug=False)

    xt = nc.dram_tensor("xt", [D, T], BF16, kind="ExternalInput").ap()
    wq = nc.dram_tensor("wq", [D, IPC], BF16, kind="ExternalInput").ap()
    wk = nc.dram_tensor("wk", [D, IPC], BF16, kind="ExternalInput").ap()
    wv = nc.dram_tensor("wv", [D, IPC], BF16, kind="ExternalInput").ap()
    wo = nc.dram_tensor("wo", [IPC, D], BF16, kind="ExternalInput").ap()
    tri = nc.dram_tensor("tri", [128, 128], BF16, kind="ExternalInput").ap()
    tri_nc = nc.dram_tensor("tri_nc", [128, 128], BF16, kind="ExternalInput").ap()
    ones = nc.dram_tensor("ones", [1, DH], BF16, kind="ExternalInput").ap()
    out = nc.dram_tensor("out", [T, D], BF16, kind="ExternalOutput").ap()

    with tile.TileContext(nc) as tc:
        with (
            tc.tile_pool(name="xt", bufs=1) as xt_pool,
            tc.tile_pool(name="w", bufs=1) as w_pool,
            tc.tile_pool(name="qkv", bufs=1) as qkv_pool,
            tc.tile_pool(name="ex", bufs=24) as ex_pool,
            tc.tile_pool(name="small", bufs=4) as small_pool,
            tc.tile_pool(name="raw", bufs=4) as raw_pool,
            tc.tile_pool(name="osb", bufs=3) as osb_pool,
            tc.tile_pool(name="ps_sc", bufs=5, space="PSUM") as ps_sc,
            tc.tile_pool(name="ps_out", bufs=2, space="PSUM") as ps_out,
            tc.tile_pool(name="ps_bc", bufs=1, space="PSUM") as ps_bc,
        ):
            # ---- load weights + masks ----
            wq_sb = w_pool.tile([128, NKC, IPC], BF16, tag="wq")
            wk_sb = w_pool.tile([128, NKC, IPC], BF16, tag="wk")
            wv_sb = w_pool.tile([128, NKC, IPC], BF16, tag="wv")
            wo_sb = w_pool.tile([IPC, D], BF16, tag="wo")
            tri_sb = w_pool.tile([128, 128], BF16, tag="tri")
            trinc_sb = w_pool.tile([128, 128], BF16, tag="trinc")
            ones_sb = w_pool.tile([1, DH], BF16, tag="ones")
            nc.sync.dma_start(wq_sb[:], wq.rearrange("(c p) m -> p c m", p=128))
            nc.sync.dma_start(wk_sb[:], wk.rearrange("(c p) m -> p c m", p=128))
            nc.sync.dma_start(wv_sb[:], wv.rearrange("(c p) m -> p c m", p=128))
            nc.sync.dma_start(wo_sb[:], wo[:])
            nc.sync.dma_start(tri_sb[:], tri[:])
            nc.sync.dma_start(trinc_sb[:], tri_nc[:])
            nc.sync.dma_start(ones_sb[:], ones[:])

            # ---- load x^T in (chunk, token-half) pieces for pipelining ----
            xt_sb = xt_pool.tile([128, NKC, T], BF16, tag="xt")
            xt_r = xt.rearrange("(c p) t -> p c t", p=128)
            for q in range(4):
                tsl = bass.ts(q, 1024)
                for c in range(NKC):
                    nc.sync.dma_start(xt_sb[:, c, tsl], xt_r[:, c, tsl])

            qt_sb = qkv_pool.tile([IPC, T], BF16, tag="qt")
            kt_sb = qkv_pool.tile([IPC, T], BF16, tag="kt")
            v_sb = qkv_pool.tile([128, HPC, TB, DH + 1], BF16, tag="v")
            attnt_sb = qkv_pool.tile([IPC, T], BF16, tag="attnt")

            # ones column of V-augmented (rowsum trick)
            nc.vector.memset(v_sb[:, :, :, DH:DH + 1], 1.0)

            # ---- stage 1: QT/KT [inner, tokens] and V [tokens, inner] ----
            for half in range(2):
                for jt in range(half * 4, half * 4 + 4):  # 512-token tiles
                    tsl = bass.ts(jt, 512)
                    for dst_sb, w_sb in ((qt_sb, wq_sb), (kt_sb, wk_sb)):
                        ps = ps_sc.tile([128, 512], F32, tag="sc")
                        for c in range(NKC):
                            nc.tensor.matmul(
                                ps[:, 0:512],
                                w_sb[:, c, :],
                                xt_sb[:, c, tsl],
                                start=(c == 0),
                                stop=(c == NKC - 1),
                            )
                        nc.scalar.copy(dst_sb[:, tsl], ps[:, 0:512])
                # V: token-major, 128-token blocks
                for tb in range(half * 16, half * 16 + 16):
                    bsl = bass.ts(tb, 128)
                    psv = ps_out.tile([128, 512], F32, tag="out")
                    for c in range(NKC):
                        nc.tensor.matmul(
                            psv[:, 0:IPC],
                            xt_sb[:, c, bsl],
                            wv_sb[:, c, :],
                            start=(c == 0),
                            stop=(c == NKC - 1),
                        )
                    nc.vector.tensor_copy(
                        v_sb[:, :, tb, 0:DH],
                        psv[:, 0:IPC].rearrange("p (h d) -> p h d", h=HPC),
                    )

            # ---- stage 2: attention per (batch, qsb), both heads fused ----
            for b in range(B):
                for qsb in range(NQSB):
                    n_kb = 4 * (qsb + 1)
                    q0 = b * N + qsb * QSB
                    out_pss = [ps_out.tile([128, 512], F32, tag="out", name="out_ps")
                               for _ in range(HPC)]
                    pv_queue = []

                    def emit_pv(kb, exs_kb):
                        j0 = max(0, (kb - 4 * qsb) * 128)
                        for h in range(HPC):
                            nc.tensor.matmul(
                                out_pss[h][0:DH + 1, j0:QSB],
                                v_sb[:, h, b * 16 + kb, :],
                                exs_kb[h][:, j0:QSB],
                                start=(kb == 0),
                                stop=(kb == n_kb - 1),
                            )

                    for kb in range(n_kb):
                        j = kb - 4 * qsb
                        jq = max(0, 128 * j)  # masked leading queries
                        k0 = b * N + kb * 128
                        scs = [ps_sc.tile([128, 512], F32, tag="sc",
                                          name="sc") for _ in range(HPC)]
                        exs = [ex_pool.tile([128, 512], BF16, tag="ex",
                                            name="ex") for _ in range(HPC)]
                        # h0/h1 use disjoint PE row groups -> concurrent;
                        # emit back-to-back with no allocs in between.
                        for h in range(HPC):
                            hsl = slice(h * DH, (h + 1) * DH)
                            nc.tensor.matmul(
                                scs[h][:, jq:QSB],
                                kt_sb[hsl, k0:k0 + 128],
                                qt_sb[hsl, q0 + jq:q0 + QSB],
                            )
                        for h in range(HPC):
                            sc, ex = scs[h], exs[h]
                            if j < 0:
                                nc.scalar.activation(
                                    ex[:], sc[:],
                                    mybir.ActivationFunctionType.Exp)
                                continue
                            nc.scalar.activation(
                                ex[:, jq:QSB], sc[:, jq:QSB],
                                mybir.ActivationFunctionType.Exp)
                            m_sb = trinc_sb if (qsb == 0 and kb == 0) else tri_sb
                            nc.gpsimd.tensor_mul(
                                ex[:, jq:jq + 128],
                                ex[:, jq:jq + 128],
                                m_sb[:],
                            )
                        # software pipeline: PV trails scores by 2 kb so the
                        # in-order PE stream never waits on a fresh exp.
                        pv_queue.append((kb, exs))
                        if len(pv_queue) > 2:
                            emit_pv(*pv_queue.pop(0))
                    for item in pv_queue:
                        emit_pv(*item)
                    # normalize: raw / rowsum (rowsum in row DH):
                    # broadcast rowsum via PE, fast-reciprocal, multiply.
                    for h in range(HPC):
                        hsl = slice(h * DH, (h + 1) * DH)
                        out_ps = out_pss[h]
                        rs = small_pool.tile([1, 512], BF16, tag="rs")
                        nc.vector.tensor_copy(rs[:], out_ps[DH:DH + 1, :])
                        bc = ps_bc.tile([DH, 512], F32, tag="bc")
                        nc.tensor.matmul(bc[:], ones_sb[:], rs[:])
                        rec = raw_pool.tile([DH, 512], F32, tag="rec64")
                        nc.vector.reciprocal_approx_fast(rec[:], bc[:])
                        nc.vector.tensor_mul(
                            attnt_sb[hsl, q0:q0 + QSB], out_ps[0:DH, :], rec[:])

            # ---- stage 3: out partial = attnT.T @ wo ----
            for tb in range(TB):
                osb = osb_pool.tile([128, D], BF16, tag="osb")
                for half in range(2):
                    pr = ps_sc.tile([128, 512], F32, tag="sc")
                    nc.tensor.matmul(
                        pr[:],
                        attnt_sb[:, bass.ts(tb, 128)],
                        wo_sb[:, bass.ts(half, 512)],
                    )
                    if half == 0:
                        nc.vector.tensor_copy(osb[:, 0:512], pr[:])
                    else:
                        nc.scalar.copy(osb[:, 512:1024], pr[:])
                nc.sync.dma_start(out[bass.ts(tb, 128), :], osb[:])

    nc.compile()
    _CACHED_NC = nc
    return nc


def make_in_maps(x, w_qkv):
    """Host-side prep: transpose/cast/slice the full inputs per core."""
    xt = np.ascontiguousarray(
        np.asarray(x, dtype=np.float32).reshape(T, D).T).astype(NP_BF16)
    w_qkv = np.asarray(w_qkv, dtype=np.float32)
    scale = DH ** -0.5
    k_idx = np.arange(128)
    tri = (k_idx[:, None] <= k_idx[None, :])
    tri_nc = tri | ((k_idx[:, None] < NONCAUSAL) & (k_idx[None, :] < NONCAUSAL))
    tri = tri.astype(NP_BF16)
    tri_nc = tri_nc.astype(NP_BF16)
    ones = np.ones((1, DH), dtype=NP_BF16)

    in_maps = []
    for c in range(N_CORES):
        h0 = c * HPC
        cols = slice(h0 * DH, (h0 + HPC) * DH)
        in_maps.append({
            "xt": xt,
            "wq": (w_qkv[:, 0 * HEADS * DH:][:, cols] * scale).astype(NP_BF16),
            "wk": w_qkv[:, 1 * HEADS * DH:][:, cols].astype(NP_BF16),
            "wv": w_qkv[:, 2 * HEADS * DH:][:, cols].astype(NP_BF16),
            "wo": None,  # filled below
            "tri": tri,
            "tri_nc": tri_nc,
            "ones": ones,
        })
    return in_maps


def run(x, mask, w_qkv, w_out, b_out, trace=False, **spmd_kwargs):
    nc = build_nc()
    in_maps = make_in_maps(x, w_qkv)
    w_out = np.asarray(w_out, dtype=np.float32)
    for c in range(N_CORES):
        in_maps[c]["wo"] = np.ascontiguousarray(
            w_out[c * IPC:(c + 1) * IPC, :]).astype(NP_BF16)
    res = run_bass_kernel_spmd(
        nc, in_maps, core_ids=list(range(N_CORES)), trace=trace, **spmd_kwargs)
    partial = np.zeros((T, D), dtype=np.float32)
    for c in range(N_CORES):
        partial += res.results[c]["out"].astype(np.float32)
    partial += np.asarray(b_out, dtype=np.float32)[None, :]
    return partial.reshape(B, N, D), res


def _axon_reset():
    """Recover a wedged axon-tunneled device (best effort)."""
    try:
        import ctypes
        import jax
        jax.devices()
        lib = ctypes.CDLL("/opt/axon/libaxon_pjrt.so")
        lib.axon_reset.restype = ctypes.c_int64
        lib.axon_reset()
    except Exception:
        pass


def kernel(x, mask, w_qkv, w_out, b_out):
    try:
        out, _ = run(x, mask, w_qkv, w_out, b_out, trace=False)
    except Exception:
        _axon_reset()
        out, _ = run(x, mask, w_qkv, w_out, b_out, trace=False)
    return out


# revision 33
# speedup vs baseline: 1.0266x; 1.0266x over previous
"""Sparse (causal + noncausal-prefix) attention on 8 TRN2 NeuronCores.

Sharding: 2 batches x 16 heads = 32 (b,h) pairs, 2 heads per core
(head-parallel). Each core receives the full x (pre-transposed to
[d, tokens] bf16 on host), its 2 heads' slices of w_qkv, and its 128
rows of w_out; it computes QKV projection, causal attention (scores
kept transposed: [keys, queries], softmax without max-subtraction --
inputs are N(0,~0.4) so exp never overflows; row-sums obtained by
appending a ones-column to V in the PV matmul), and a row-sharded
output projection. Host sums the 8 partial outputs and adds b_out.
"""
import numpy as np
import ml_dtypes

import concourse.bass as bass
import concourse.tile as tile
from concourse import bacc, mybir
from concourse.bass_utils import run_bass_kernel_spmd

# Problem constants (hardcoded per contract).
B = 2
N = 2048
D = 1024
HEADS = 16
DH = 64
NONCAUSAL = 64
T = B * N          # 4096 tokens
N_CORES = 8
HPC = HEADS // N_CORES   # heads per core = 2
IPC = HPC * DH           # inner dims per core = 128
NKC = D // 128           # contraction chunks = 8
TB = T // 128            # 128-token blocks = 32
QSB = 512                # query superblock
NQSB = N // QSB          # 4 per batch

BF16 = mybir.dt.bfloat16
F32 = mybir.dt.float32
NP_BF16 = ml_dtypes.bfloat16

_CACHED_NC = None


def build_nc():
    global _CACHED_NC
    if _CACHED_NC is not None:
        return _CACHED_NC
    nc = bacc.Bacc("TRN2", target_bir_lowering=False, debug=False)

    xt = nc.dram_tensor("xt", [D, T], BF16, kind="ExternalInput").ap()
    wq = nc.dram_tensor("wq", [D, IPC], BF16, kind="ExternalInput").ap()
    wk = nc.dram_tensor("wk", [D, IPC], BF16, kind="ExternalInput").ap()
    wv = nc.dram_tensor("wv", [D, IPC], BF16, kind="ExternalInput").ap()
    wo = nc.dram_tensor("wo", [IPC, D], BF16, kind="ExternalInput").ap()
    tri = nc.dram_tensor("tri", [128, 128], BF16, kind="ExternalInput").ap()
    tri_nc = nc.dram_tensor("tri_nc", [128, 128], BF16, kind="ExternalInput").ap()
    ones = nc.dram_tensor("ones", [1, DH], BF16, kind="ExternalInput").ap()
    out = nc.dram_tensor("out", [T, D], BF16, kind="ExternalOutput").ap()

    with tile.TileContext(nc) as tc:
        with (
            tc.tile_pool(name="xt", bufs=1) as xt_pool,
            tc.tile_pool(name="w", bufs=1) as w_pool,
            tc.tile_pool(name="qkv", bufs=1) as qkv_pool,
            tc.tile_pool(name="ex", bufs=24) as ex_pool,
            tc.tile_pool(name="small", bufs=4) as small_pool,
            tc.tile_pool(name="raw", bufs=4) as raw_pool,
            tc.tile_pool(name="osb", bufs=3) as osb_pool,
            tc.tile_pool(name="ps_sc", bufs=5, space="PSUM") as ps_sc,
            tc.tile_pool(name="ps_out", bufs=2, space="PSUM") as ps_out,
            tc.tile_pool(name="ps_bc", bufs=1, space="PSUM") as ps_bc,
        ):
            # ---- load weights + masks ----
            wq_sb = w_pool.tile([128, NKC, IPC], BF16, tag="wq")
            wk_sb = w_pool.tile([128, NKC, IPC], BF16, tag="wk")
            wv_sb = w_pool.tile([128, NKC, IPC], BF16, tag="wv")
            wo_sb = w_pool.tile([IPC, D], BF16, tag="wo")
            tri_sb = w_pool.tile([128, 128], BF16, tag="tri")
            trinc_sb = w_pool.tile([128, 128], BF16, tag="trinc")
            ones_sb = w_pool.tile([1, DH], BF16, tag="ones")
            nc.sync.dma_start(wq_sb[:], wq.rearrange("(c p) m -> p c m", p=128))
            nc.sync.dma_start(wk_sb[:], wk.rearrange("(c p) m -> p c m", p=128))
            nc.sync.dma_start(wv_sb[:], wv.rearrange("(c p) m -> p c m", p=128))
            nc.sync.dma_start(wo_sb[:], wo[:])
            nc.sync.dma_start(tri_sb[:], tri[:])
            nc.sync.dma_start(trinc_sb[:], tri_nc[:])
            nc.sync.dma_start(ones_sb[:], ones[:])

            # ---- load x^T in (chunk, token-half) pieces for pipelining ----
            xt_sb = xt_pool.tile([128, NKC, T], BF16, tag="xt")
            xt_r = xt.rearrange("(c p) t -> p c t", p=128)
            for q in range(4):
                tsl = bass.ts(q, 1024)
                for c in range(NKC):
                    nc.sync.dma_start(xt_sb[:, c, tsl], xt_r[:, c, tsl])

            qt_sb = qkv_pool.tile([IPC, T], BF16, tag="qt")
            kt_sb = qkv_pool.tile([IPC, T], BF16, tag="kt")
            v_sb = qkv_pool.tile([128, HPC, TB, DH + 1], BF16, tag="v")
            attnt_sb = qkv_pool.tile([IPC, T], BF16, tag="attnt")

            # ones column of V-augmented (rowsum trick)
            nc.vector.memset(v_sb[:, :, :, DH:DH + 1], 1.0)

            # ---- stage 1: QT/KT [inner, tokens] and V [tokens, inner],
            # interleaved per 512-token quarter so attention can start early.
            for jt in range(8):
                tsl = bass.ts(jt, 512)
                for dst_sb, w_sb in ((qt_sb, wq_sb), (kt_sb, wk_sb)):
                    ps = ps_sc.tile([128, 512], F32, tag="sc", name="ps")
                    for c in range(NKC):
                        nc.tensor.matmul(
                            ps[:],
                            w_sb[:, c, :],
                            xt_sb[:, c, tsl],
                            start=(c == 0),
                            stop=(c == NKC - 1),
                        )
                    nc.scalar.copy(dst_sb[:, tsl], ps[:])
                for tb in range(jt * 4, jt * 4 + 4):
                    bsl = bass.ts(tb, 128)
                    psv = ps_out.tile([128, 512], F32, tag="out", name="psv")
                    for c in range(NKC):
                        nc.tensor.matmul(
                            psv[:, 0:IPC],
                            xt_sb[:, c, bsl],
                            wv_sb[:, c, :],
                            start=(c == 0),
                            stop=(c == NKC - 1),
                        )
                    nc.vector.tensor_copy(
                        v_sb[:, :, tb, 0:DH],
                        psv[:, 0:IPC].rearrange("p (h d) -> p h d", h=HPC),
                    )

            done = []

            def emit_proj(task):
                tb0 = task[0] * 16 + task[1] * 4
                for tb in range(tb0, tb0 + 4):
                    osb = osb_pool.tile([128, D], BF16, tag="osb", name="osb")
                    for half in range(2):
                        pr = ps_sc.tile([128, 512], F32, tag="sc", name="pr")
                        nc.tensor.matmul(
                            pr[:],
                            attnt_sb[:, bass.ts(tb, 128)],
                            wo_sb[:, bass.ts(half, 512)],
                        )
                        if half == 0:
                            nc.vector.tensor_copy(osb[:, 0:512], pr[:])
                        else:
                            nc.scalar.copy(osb[:, 512:1024], pr[:])
                    nc.sync.dma_start(out[bass.ts(tb, 128), :], osb[:])

            # ---- stage 2: attention per (batch, qsb), both heads fused ----
            for b in range(B):
                for qsb in range(NQSB):
                    n_kb = 4 * (qsb + 1)
                    q0 = b * N + qsb * QSB
                    out_pss = [ps_out.tile([128, 512], F32, tag="out", name="out_ps")
                               for _ in range(HPC)]
                    pv_queue = []

                    def emit_pv(kb, exs_kb):
                        j0 = max(0, (kb - 4 * qsb) * 128)
                        for h in range(HPC):
                            nc.tensor.matmul(
                                out_pss[h][0:DH + 1, j0:QSB],
                                v_sb[:, h, b * 16 + kb, :],
                                exs_kb[h][:, j0:QSB],
                                start=(kb == 0),
                                stop=(kb == n_kb - 1),
                            )

                    for kb in range(n_kb):
                        j = kb - 4 * qsb
                        jq = max(0, 128 * j)  # masked leading queries
                        k0 = b * N + kb * 128
                        scs = [ps_sc.tile([128, 512], F32, tag="sc",
                                          name="sc") for _ in range(HPC)]
                        exs = [ex_pool.tile([128, 512], BF16, tag="ex",
                                            name="ex") for _ in range(HPC)]
                        # h0/h1 use disjoint PE row groups -> concurrent;
                        # emit back-to-back with no allocs in between.
                        for h in range(HPC):
                            hsl = slice(h * DH, (h + 1) * DH)
                            nc.tensor.matmul(
                                scs[h][:, jq:QSB],
                                kt_sb[hsl, k0:k0 + 128],
                                qt_sb[hsl, q0 + jq:q0 + QSB],
                            )
                        for h in range(HPC):
                            sc, ex = scs[h], exs[h]
                            if j < 0:
                                nc.scalar.activation(
                                    ex[:], sc[:],
                                    mybir.ActivationFunctionType.Exp)
                                continue
                            nc.scalar.activation(
                                ex[:, jq:QSB], sc[:, jq:QSB],
                                mybir.ActivationFunctionType.Exp)
                            m_sb = trinc_sb if (qsb == 0 and kb == 0) else tri_sb
                            nc.gpsimd.tensor_mul(
                                ex[:, jq:jq + 128],
                                ex[:, jq:jq + 128],
                                m_sb[:],
                            )
                        # software pipeline: PV trails scores by 2 kb so the
                        # in-order PE stream never waits on a fresh exp.
                        pv_queue.append((kb, exs))
                        if len(pv_queue) > 2:
                            emit_pv(*pv_queue.pop(0))
                    for item in pv_queue:
                        emit_pv(*item)
                    # normalize: raw / rowsum (rowsum in row DH):
                    # broadcast rowsum via PE, fast-reciprocal, multiply.
                    for h in range(HPC):
                        hsl = slice(h * DH, (h + 1) * DH)
                        out_ps = out_pss[h]
                        rs = small_pool.tile([1, 512], BF16, tag="rs")
                        nc.vector.tensor_copy(rs[:], out_ps[DH:DH + 1, :])
                        bc = ps_bc.tile([DH, 512], F32, tag="bc")
                        nc.tensor.matmul(bc[:], ones_sb[:], rs[:])
                        rec = raw_pool.tile([DH, 512], F32, tag="rec64")
                        nc.vector.reciprocal_approx_fast(rec[:], bc[:])
                        nc.vector.tensor_mul(
                            attnt_sb[hsl, q0:q0 + QSB], out_ps[0:DH, :], rec[:])
                    done.append((b, qsb))
                    if len(done) > 2:
                        emit_proj(done[-3])


            # flush the last two deferred projection groups
            for task in done[-2:]:
                emit_proj(task)
    nc.compile()
    _CACHED_NC = nc
    return nc


def make_in_maps(x, w_qkv):
    """Host-side prep: transpose/cast/slice the full inputs per core."""
    xt = np.ascontiguousarray(
        np.asarray(x, dtype=np.float32).reshape(T, D).T).astype(NP_BF16)
    w_qkv = np.asarray(w_qkv, dtype=np.float32)
    scale = DH ** -0.5
    k_idx = np.arange(128)
    tri = (k_idx[:, None] <= k_idx[None, :])
    tri_nc = tri | ((k_idx[:, None] < NONCAUSAL) & (k_idx[None, :] < NONCAUSAL))
    tri = tri.astype(NP_BF16)
    tri_nc = tri_nc.astype(NP_BF16)
    ones = np.ones((1, DH), dtype=NP_BF16)

    in_maps = []
    for c in range(N_CORES):
        h0 = c * HPC
        cols = slice(h0 * DH, (h0 + HPC) * DH)
        in_maps.append({
            "xt": xt,
            "wq": (w_qkv[:, 0 * HEADS * DH:][:, cols] * scale).astype(NP_BF16),
            "wk": w_qkv[:, 1 * HEADS * DH:][:, cols].astype(NP_BF16),
            "wv": w_qkv[:, 2 * HEADS * DH:][:, cols].astype(NP_BF16),
            "wo": None,  # filled below
            "tri": tri,
            "tri_nc": tri_nc,
            "ones": ones,
        })
    return in_maps


def run(x, mask, w_qkv, w_out, b_out, trace=False, **spmd_kwargs):
    nc = build_nc()
    in_maps = make_in_maps(x, w_qkv)
    w_out = np.asarray(w_out, dtype=np.float32)
    for c in range(N_CORES):
        in_maps[c]["wo"] = np.ascontiguousarray(
            w_out[c * IPC:(c + 1) * IPC, :]).astype(NP_BF16)
    res = run_bass_kernel_spmd(
        nc, in_maps, core_ids=list(range(N_CORES)), trace=trace, **spmd_kwargs)
    partial = np.zeros((T, D), dtype=np.float32)
    for c in range(N_CORES):
        partial += res.results[c]["out"].astype(np.float32)
    partial += np.asarray(b_out, dtype=np.float32)[None, :]
    return partial.reshape(B, N, D), res


def _axon_reset():
    """Recover a wedged axon-tunneled device (best effort)."""
    try:
        import ctypes
        import jax
        jax.devices()
        lib = ctypes.CDLL("/opt/axon/libaxon_pjrt.so")
        lib.axon_reset.restype = ctypes.c_int64
        lib.axon_reset()
    except Exception:
        pass


def kernel(x, mask, w_qkv, w_out, b_out):
    try:
        out, _ = run(x, mask, w_qkv, w_out, b_out, trace=False)
    except Exception:
        _axon_reset()
        out, _ = run(x, mask, w_qkv, w_out, b_out, trace=False)
    return out

